# revision 1
# baseline (speedup 1.0000x reference)
"""Trainium2 Bass kernel for nn_Net_71270687310327 (scatter_memory).

Computation (see reference):
  - keys = (timings+1)*512 + slot_index, with argmin(surprise*0.9) slot's key
    overridden to its slot index (forces rank 0, stable-sort tiebreak exact).
  - rank[b,m] = #{m' : key[b,m'] < key[b,m]}  (all keys distinct)
  - pred_in = [sorted memory rows | timing bits], fed to a 4-layer MLP.

Sharding: W0 row-sharded over 8 cores by slot-rank range (64 ranks/core,
17024 rows of W0 each, fully contiguous HBM reads). Each core gathers only
its 64 ranks' memory rows (dma_gather), computes a partial h = pred_in @ W0
contribution, AllReduce over the 8 cores, then every core redundantly runs
the small W1/W2/Wout layers.

The same program runs on all 8 cores (SPMD); all per-core differences are
carried by per-core input constants (W0 shard, rank-range constants).
"""

import sys, os

sys.path.insert(0, "/opt/trn_rl_repo")

import numpy as np

import concourse.bass as bass
import concourse.bacc as bacc
import concourse.mybir as mybir
from concourse import tile
from concourse import bass_utils

class _SkipRest(Exception):
    pass


F32 = mybir.dt.float32
I16 = mybir.dt.int16
ALU = mybir.AluOpType
ACTF = mybir.ActivationFunctionType

B, M, V, H, TD = 32, 512, 256, 1024, 10
NC = 8
RPC = M // NC            # 64 ranks per core
MEMROWS = B * M          # 16384
MEMP = MEMROWS + B       # 16416 (gather source rows: memory rows + x rows)
NKT = RPC * V // 128     # 128 mem k-tiles per core
NBT = RPC * TD // 128    # 5 bits k-tiles per core
W0S_ROWS = RPC * V + RPC * TD  # 17024
NIDX = RPC * B           # 2048 gather indices per core


def build_program(stage="full"):
    nkt_lim = NKT + NBT
    if stage.startswith("parth") and stage != "parth":
        nkt_lim = int(stage[5:])
        stage = "parth"
    lvl = {"idx": 0, "tk": 1, "parth": 2, "full": 3}[stage]
    nc = bacc.Bacc(
        "TRN2",
        target_bir_lowering=False,
        debug=False,
        enable_asserts=False,
        num_devices=NC,
    )

    def din(name, shape, dtype=F32):
        return nc.dram_tensor(name, list(shape), dtype, kind="ExternalInput").ap()

    mem_plus = din("mem_plus", (MEMP, V))
    timings = din("timings", (B, M))
    msur = din("msur", (B, M))
    w0s = din("W0s", (W0S_ROWS, H))
    w1 = din("W1", (H, H))
    w2 = din("W2", (H, H))
    wout = din("Wout", (H, V))
    b0r = din("b0r", (B, H))
    b1r = din("b1r", (B, H))
    b2r = din("b2r", (B, H))
    boutr = din("boutr", (B, V))
    c_eye = din("c_eye", (128, 128))
    c_esel = din("c_esel", (B, B * 128))
    c_iota = din("c_iota512", (B, M))
    c_iotam = din("c_iotam", (128, 4))
    c_rrow = din("c_rrow", (128, RPC))
    c_sel16 = din("c_sel16", (1, 16 * 128))
    c_amask = din("c_amask", (128, 128))
    c_coff = din("c_coff", (128, 128))
    c_rtd = din("c_rtd", (RPC, NBT * TD * 128))

    out = nc.dram_tensor("out", [B, V], F32, kind="ExternalOutput").ap()
    dbg = (nc.dram_tensor("dbg", [128, 256], F32, kind="ExternalOutput").ap()
           if stage != "full" else None)

    with tile.TileContext(nc) as tc:
        with (
            tc.tile_pool(name="const", bufs=1) as constp,
            tc.tile_pool(name="state", bufs=1) as state,
            tc.tile_pool(name="wres", bufs=1) as wres,
            tc.tile_pool(name="krep", bufs=2) as krepp,
            tc.tile_pool(name="pt", bufs=8) as ptp,
            tc.tile_pool(name="w0t", bufs=6) as w0p,
            tc.tile_pool(name="pk", bufs=1, space="PSUM") as pkp,
            tc.tile_pool(name="pflat", bufs=1, space="PSUM") as pflatp,
            tc.tile_pool(name="psort", bufs=1, space="PSUM") as psortp,
            tc.tile_pool(name="ptr", bufs=2, space="PSUM") as ptrp,
            tc.tile_pool(name="ph", bufs=1, space="PSUM") as php,
            tc.tile_pool(name="dram", bufs=1, space="DRAM") as dramp,
        ):
            # ---- constants / small state into SBUF
            def load(pool, ap):
                t = pool.tile(list(ap.shape), ap.dtype, tag=f"ld_{ap.tensor.name}")
                nc.sync.dma_start(t[:], ap)
                return t

            eye = load(constp, c_eye)
            esel = load(constp, c_esel)
            iota = load(constp, c_iota)
            iotam = load(constp, c_iotam)
            rrow = load(constp, c_rrow)
            sel16 = load(constp, c_sel16)
            amask = load(constp, c_amask)
            coff = load(constp, c_coff)
            rtd = load(constp, c_rtd)
            b0s = load(constp, b0r)
            b1s = load(constp, b1r)
            b2s = load(constp, b2r)
            bouts = load(constp, boutr)
            t_sb = load(state, timings)
            ms_sb = load(state, msur)

            # resident output-layer weights; W1/W2 stream through the k-tile pool
            wos = wres.tile([128, 8 * V], F32, tag="wos")
            for kt in range(8):
                nc.sync.dma_start(wos[:, kt * V:(kt + 1) * V], wout[kt * 128:(kt + 1) * 128, :])

            # ---- stage A: keys -------------------------------------------
            msur2 = state.tile([B, M], F32, tag="msur2")
            nc.vector.tensor_scalar(msur2[:], ms_sb[:], 0.9, None, ALU.mult)
            minv = state.tile([B, 1], F32, tag="minv")
            nc.vector.tensor_reduce(minv[:], msur2[:], axis=mybir.AxisListType.X, op=ALU.min)
            mask = state.tile([B, M], mybir.dt.uint8, tag="mask")
            nc.vector.tensor_scalar(mask[:], msur2[:], minv[:], None, ALU.is_equal)
            cand = state.tile([B, M], F32, tag="cand")
            nc.vector.memset(cand[:], 1.0e9)
            nc.vector.copy_predicated(cand[:], mask[:], iota[:])
            idx0 = state.tile([B, 1], F32, tag="idx0")
            nc.vector.tensor_reduce(idx0[:], cand[:], axis=mybir.AxisListType.X, op=ALU.min)

            keys = state.tile([B, M], F32, tag="keys")
            # (t+1)*512 + m  =  t*512 + 512 + m
            nc.vector.tensor_scalar(keys[:], t_sb[:], 512.0, 512.0, ALU.mult, ALU.add)
            nc.vector.tensor_tensor(keys[:], keys[:], iota[:], ALU.add)
            mask2 = state.tile([B, M], mybir.dt.uint8, tag="mask2")
            nc.vector.tensor_scalar(mask2[:], iota[:], idx0[:], None, ALU.is_equal)
            nc.vector.copy_predicated(keys[:], mask2[:], iota[:])

            # ---- keysT via PE transpose ----------------------------------
            keysT = state.tile([128, 4 * B], F32, tag="keysT")
            for mt in range(4):
                ptt = ptrp.tile([128, 128], F32, tag="pm")
                nc.tensor.transpose(ptt[:, 0:B], keys[:, mt * 128:(mt + 1) * 128], eye[0:B, 0:B])
                nc.scalar.activation(keysT[:, mt * B:(mt + 1) * B], ptt[:, 0:B], ACTF.Copy)

            # ---- ranks, P^T, order/sorted extraction ---------------------
            rank_sb = state.tile([128, 4 * B], F32, tag="rank")
            scratch = state.tile([128, M], F32, tag="scratch")
            flat = state.tile([1, NIDX], F32, tag="flat")
            psort_t = psortp.tile([RPC, B], F32, tag="psort")
            for g in range(4):
                pflat_t = pflatp.tile([1, 8 * RPC], F32, tag="pflat")
                for b8 in range(8):
                    b = g * 8 + b8
                    pk_t = pkp.tile([128, M], F32, tag="pkrep")
                    nc.tensor.matmul(pk_t[:], esel[:, b * 128:(b + 1) * 128], keys[:],
                                     start=True, stop=True)
                    krep = krepp.tile([128, M], F32, tag="krep")
                    nc.scalar.activation(krep[:], pk_t[:], ACTF.Copy)
                    for mt in range(4):
                        nc.vector.tensor_scalar(
                            scratch[:], krep[:], keysT[:, mt * B + b:mt * B + b + 1], None,
                            ALU.is_lt, ALU.add,
                            accum_out=rank_sb[:, b * 4 + mt:b * 4 + mt + 1])
                    pts = []
                    for mt in range(4):
                        pt_t = ptp.tile([128, RPC], F32, tag="pt")
                        nc.vector.tensor_scalar(
                            pt_t[:], rrow[:], rank_sb[:, b * 4 + mt:b * 4 + mt + 1], None,
                            ALU.is_equal)
                        pts.append(pt_t)
                    for mt in range(4):
                        nc.tensor.matmul(
                            pflat_t[0:1, b8 * RPC:(b8 + 1) * RPC],
                            iotam[:, mt:mt + 1], pts[mt][:],
                            start=(mt == 0), stop=(mt == 3))
                        nc.tensor.matmul(
                            psort_t[0:RPC, b:b + 1],
                            pts[mt][:], keysT[:, mt * B + b:mt * B + b + 1],
                            start=(mt == 0), stop=(mt == 3))
                nc.scalar.activation(flat[0:1, g * 512:(g + 1) * 512], pflat_t[:], ACTF.Copy)

            # ---- bits from sorted keys -----------------------------------
            # binary decomposition of sorted key (< 2^19); timing bit d of t
            # is key bit d+9.  u_all[:, d*B:(d+1)*B] = bit (d+9) of key.
            skT = state.tile([RPC, B], F32, tag="skT")
            nc.scalar.activation(skT[:], psort_t[:], ACTF.Copy)
            rem = state.tile([RPC, B], F32, tag="rem")
            nc.vector.tensor_copy(rem[:], skT[:])
            u_all = state.tile([RPC, TD * B], F32, tag="u_all")
            tmpu = state.tile([RPC, B], F32, tag="tmpu")
            for j in range(18, 8, -1):
                d = j - 9
                ud = u_all[:, d * B:(d + 1) * B]
                nc.vector.tensor_scalar(ud, rem[:], float(2 ** j), None, ALU.is_ge)
                nc.vector.tensor_scalar(tmpu[:], ud, float(2 ** j), None, ALU.mult)
                nc.vector.tensor_tensor(rem[:], rem[:], tmpu[:], ALU.subtract)
            # bits_sb[t][p, b] = u_{d(p)}[r(p), b] via selection matmuls
            bits_sb = state.tile([128, NBT * B], F32, tag="bits")
            for t in range(NBT):
                pb = ptrp.tile([128, 128], F32, tag="pm")
                for d in range(TD):
                    nc.tensor.matmul(pb[:, 0:B],
                                     rtd[:, (t * TD + d) * 128:(t * TD + d + 1) * 128],
                                     u_all[:, d * B:(d + 1) * B],
                                     start=(d == 0), stop=(d == TD - 1))
                nc.scalar.activation(bits_sb[:, t * B:(t + 1) * B], pb[:, 0:B], ACTF.Copy)

            # ---- gather indices ------------------------------------------
            pidx_t = ptrp.tile([128, 128], F32, tag="pm")
            flat_v = flat.rearrange("p (n s) -> p n s", s=16)
            for k in range(16):
                nc.tensor.matmul(pidx_t[:], sel16[0:1, k * 128:(k + 1) * 128],
                                 flat_v[:, :, k], start=(k == 0), stop=(k == 15))
            tmpidx = state.tile([128, 128], F32, tag="tmpidx")
            nc.vector.tensor_tensor(tmpidx[:], pidx_t[:], amask[:], ALU.mult)
            idx_sb = state.tile([128, 128], I16, tag="idx")
            nc.vector.tensor_tensor(idx_sb[:], tmpidx[:], coff[:], ALU.add)

            if stage == "idx":
                nc.vector.tensor_copy(tmpidx[:], idx_sb[:])
                nc.sync.dma_start(dbg[:, 0:128], tmpidx[:])
                nc.sync.dma_start(dbg[:, 128:256], bits_sb[:, 0:128])
            do_rest = lvl >= 1
            try:
              if not do_rest:
                  raise _SkipRest
              # ---- gather + transpose to pred_in^T tiles -------------------
              G = state.tile([128, 16 * V], F32, tag="G")
              nc.gpsimd.dma_gather(
                  out_ap=G.rearrange("p (c e) -> p c e", e=V),
                  in_ap=mem_plus,
                  idxs_ap=idx_sb[:],
                  num_idxs=NIDX,
                  num_idxs_reg=NIDX,
                  elem_size=V,
                  single_packet=False,
              )
              T_all = state.tile([128, 16 * V], F32, tag="T_all")
              for c in range(16):
                  for hh in range(2):
                      off = c * V + hh * 128
                      pt2 = ptrp.tile([128, 128], F32, tag="pm")
                      nc.tensor.transpose(pt2[:], G[:, off:off + 128], eye[:])
                      nc.scalar.activation(T_all[:, off:off + 128], pt2[:], ACTF.Copy)

              # ---- repack transposed tiles to k-tile-major contiguous ------
              # T_all col = 256*cb + 128*h + 64*b2 + r  ->  TK col = 64*r + 32*h + 2*cb + b2
              TK = state.tile([128, 16 * V], F32, tag="TK")
              t_in = T_all.rearrange("p (cb h b2 r) -> p r h cb b2", cb=16, h=2, b2=2, r=RPC)
              tk_out = TK.rearrange("p (r h cb b2) -> p r h cb b2", r=RPC, h=2, cb=16, b2=2)
              nc.vector.tensor_copy(tk_out[:], t_in[:])

              if stage == "tk":
                  nc.sync.dma_start(dbg[:, 0:256], TK[:, 0:256])
              if lvl < 2:
                  raise _SkipRest
              # ---- main matmul: partial h = pred_in_shard @ W0_shard -------
              ph_t = php.tile([B, H], F32, tag="ph")
              for kt in range(nkt_lim):
                  w0t = w0p.tile([128, H], F32, tag="w0t")
                  nc.sync.dma_start(w0t[:], w0s[kt * 128:(kt + 1) * 128, :])
                  if kt < NKT:
                      lhsT = TK[:, kt * B:(kt + 1) * B]
                  else:
                      tb = kt - NKT
                      lhsT = bits_sb[:, tb * B:(tb + 1) * B]
                  last = kt == nkt_lim - 1
                  nc.tensor.matmul(ph_t[:, 0:512], lhsT, w0t[:, 0:512],
                                   start=(kt == 0), stop=last)
                  nc.tensor.matmul(ph_t[:, 512:1024], lhsT, w0t[:, 512:1024],
                                   start=(kt == 0), stop=last)

              # ---- AllReduce partial h over the 8 cores --------------------
              part_h = state.tile([B, H], F32, tag="part_h")
              nc.vector.tensor_copy(part_h[:], ph_t[:])
              if stage == "parth":
                  nc.sync.dma_start(dbg[0:B, 0:256], part_h[:, 0:256])
              if lvl < 3:
                  raise _SkipRest
              cc_in = dramp.tile([B, H], F32, tag="cc_in")
              cc_out = dramp.tile([B, H], F32, tag="cc_out")
              nc.sync.dma_start(cc_in[:], part_h[:])
              nc.gpsimd.collective_compute(
                  "AllReduce", ALU.add,
                  replica_groups=[list(range(NC))],
                  ins=[cc_in.opt()],
                  outs=[cc_out.opt()],
              )
              h_sb = state.tile([B, H], F32, tag="h_sb")
              nc.sync.dma_start(h_sb[:], cc_out[:])

              # ---- dense layers (replicated on every core) -----------------
              nc.vector.tensor_tensor(h_sb[:], h_sb[:], b0s[:], ALU.add)
              nc.vector.tensor_scalar(h_sb[:], h_sb[:], 0.0, None, ALU.max)

              def dense(h_in, w_dram, w_sb, bias_sb, n_out, relu, tag):
                  hT = state.tile([128, 8 * B], F32, tag=f"hT_{tag}")
                  for kt in range(8):
                      ptt = ptrp.tile([128, 128], F32, tag="pm")
                      nc.tensor.transpose(ptt[:, 0:B], h_in[:, kt * 128:(kt + 1) * 128], eye[0:B, 0:B])
                      nc.scalar.activation(hT[:, kt * B:(kt + 1) * B], ptt[:, 0:B], ACTF.Copy)
                  pho = php.tile([B, n_out], F32, tag="ph")
                  for kt in range(8):
                      if w_dram is not None:
                          wt = w0p.tile([128, H], F32, tag="w0t")
                          nc.sync.dma_start(wt[:, 0:n_out], w_dram[kt * 128:(kt + 1) * 128, :])
                      else:
                          wt = None
                      for j0 in range(0, n_out, 512):
                          jn = min(512, n_out - j0)
                          rhs = (wt[:, j0:j0 + jn] if wt is not None
                                 else w_sb[:, kt * n_out + j0:kt * n_out + j0 + jn])
                          nc.tensor.matmul(
                              pho[:, j0:j0 + jn], hT[:, kt * B:(kt + 1) * B], rhs,
                              start=(kt == 0), stop=(kt == 7))
                  h_next = state.tile([B, n_out], F32, tag=f"h_{tag}")
                  nc.vector.tensor_tensor(h_next[:], pho[:], bias_sb[:], ALU.add)
                  if relu:
                      nc.vector.tensor_scalar(h_next[:], h_next[:], 0.0, None, ALU.max)
                  return h_next

              h1 = dense(h_sb, w1, None, b1s, H, True, "l1")
              h2 = dense(h1, w2, None, b2s, H, True, "l2")
              logits = dense(h2, None, wos, bouts, V, False, "lo")
              nc.sync.dma_start(out, logits[:])
            except _SkipRest:
                pass

    nc.compile()
    return nc


def make_in_maps(inputs):
    x = np.asarray(inputs["x"], np.float32)
    memory = np.asarray(inputs["memory"], np.float32)
    timings = np.asarray(inputs["memory_timings"], np.float32)
    msur = np.asarray(inputs["memory_surprise"], np.float32)
    W0 = np.asarray(inputs["W0"], np.float32)
    W1 = np.asarray(inputs["W1"], np.float32)
    W2 = np.asarray(inputs["W2"], np.float32)
    Wout = np.asarray(inputs["Wout"], np.float32)
    b0 = np.asarray(inputs["b0"], np.float32)
    b1 = np.asarray(inputs["b1"], np.float32)
    b2 = np.asarray(inputs["b2"], np.float32)
    bout = np.asarray(inputs["bout"], np.float32)

    mem_plus = np.concatenate([memory.reshape(MEMROWS, V), x], 0)

    # shared constants
    eye = np.eye(128, dtype=np.float32)
    esel = np.zeros((B, B * 128), np.float32)
    for b in range(B):
        esel[b, b * 128:(b + 1) * 128] = 1.0
    iota512 = np.broadcast_to(np.arange(M, dtype=np.float32), (B, M)).copy()
    iotam = np.empty((128, 4), np.float32)
    for mt in range(4):
        iotam[:, mt] = np.arange(128) + mt * 128
    sel16 = np.zeros((1, 16 * 128), np.float32)
    for k in range(16):
        p = np.arange(128)
        sel16[0, k * 128:(k + 1) * 128] = (p % 16 == k)
    rtd = np.zeros((RPC, NBT * TD * 128), np.float32)
    for t in range(NBT):
        for p in range(128):
            l = t * 128 + p
            rp, dp = l // TD, l % TD
            rtd[rp, (t * TD + dp) * 128 + p] = 1.0

    shared = {
        "mem_plus": mem_plus,
        "timings": timings,
        "msur": msur,
        "W1": W1, "W2": W2, "Wout": Wout,
        "b0r": np.broadcast_to(b0, (B, H)).copy(),
        "b1r": np.broadcast_to(b1, (B, H)).copy(),
        "b2r": np.broadcast_to(b2, (B, H)).copy(),
        "boutr": np.broadcast_to(bout, (B, V)).copy(),
        "c_eye": eye, "c_esel": esel, "c_iota512": iota512,
        "c_iotam": iotam, "c_sel16": sel16, "c_rtd": rtd,
    }

    in_maps = []
    p = np.arange(128)
    f = np.arange(128)
    ii = 16 * f[None, :] + (p % 16)[:, None]   # [128,128] linear gather positions
    bb = ii // RPC
    rr = ii % RPC
    for core in range(NC):
        w0shard = np.concatenate(
            [W0[core * RPC * V:(core + 1) * RPC * V],
             W0[M * V + core * RPC * TD: M * V + (core + 1) * RPC * TD]], 0)
        rrowc = np.broadcast_to(
            np.arange(core * RPC, (core + 1) * RPC, dtype=np.float32), (128, RPC)).copy()
        am = np.ones((128, 128), np.float32)
        co = (512.0 * bb).astype(np.float32)
        if core == 0:
            r0 = rr == 0
            am[r0] = 0.0
            co[r0] = (MEMROWS + bb)[r0]
        m = dict(shared)
        m["W0s"] = np.ascontiguousarray(w0shard)
        m["c_rrow"] = rrowc
        m["c_amask"] = am
        m["c_coff"] = co
        in_maps.append(m)
    return in_maps


_NC_CACHE = None


def kernel(**inputs) -> np.ndarray:
    global _NC_CACHE
    if _NC_CACHE is None:
        _NC_CACHE = build_program()
    nc = _NC_CACHE
    in_maps = make_in_maps(inputs)
    res = bass_utils.run_bass_kernel_spmd(nc, in_maps, core_ids=list(range(NC)))
    return np.asarray(res.results[0]["out"], np.float32)


if __name__ == "__main__":
    np.random.seed(0)
    build_program()
    print("build OK")



# revision 8
# speedup vs baseline: 1.2922x; 1.2922x over previous
"""Trainium2 Bass kernel for nn_Net_71270687310327 (scatter_memory).

Computation (see reference):
  - keys = (timings+1)*512 + slot_index, with argmin(surprise*0.9) slot's key
    overridden to its slot index (forces rank 0, stable-sort tiebreak exact).
  - rank[b,m] = #{m' : key[b,m'] < key[b,m]}  (all keys distinct)
  - pred_in = [sorted memory rows | timing bits], fed to a 4-layer MLP.

Sharding: W0 row-sharded over 8 cores by slot-rank range (64 ranks/core,
17024 rows of W0 each, fully contiguous HBM reads). Each core gathers only
its 64 ranks' memory rows (dma_gather), computes a partial h = pred_in @ W0
contribution, AllReduce over the 8 cores, then every core redundantly runs
the small W1/W2/Wout layers.

The same program runs on all 8 cores (SPMD); all per-core differences are
carried by per-core input constants (W0 shard, rank-range constants).
"""

import sys, os

sys.path.insert(0, "/opt/trn_rl_repo")

import numpy as np
import ml_dtypes
BF = ml_dtypes.bfloat16

import concourse.bass as bass
import concourse.bacc as bacc
import concourse.mybir as mybir
from concourse import tile
from concourse import bass_utils

class _SkipRest(Exception):
    pass


F32 = mybir.dt.float32
BF16 = mybir.dt.bfloat16
I16 = mybir.dt.int16
ALU = mybir.AluOpType
ACTF = mybir.ActivationFunctionType

B, M, V, H, TD = 32, 512, 256, 1024, 10
NC = 8
RPC = M // NC            # 64 ranks per core
MEMROWS = B * M          # 16384
MEMP = MEMROWS + B       # 16416 (gather source rows: memory rows + x rows)
NKT = RPC * V // 128     # 128 mem k-tiles per core
NBT = RPC * TD // 128    # 5 bits k-tiles per core
W0S_ROWS = RPC * V + RPC * TD  # 17024
NIDX = RPC * B           # 2048 gather indices per core


def build_program(stage="full"):
    nkt_lim = NKT + NBT
    if stage.startswith("parth") and stage != "parth":
        nkt_lim = int(stage[5:])
        stage = "parth"
    lvl = {"idx": 0, "tk": 1, "parth": 2, "full": 3}[stage]
    nc = bacc.Bacc(
        "TRN2",
        target_bir_lowering=False,
        debug=False,
        enable_asserts=False,
        num_devices=NC,
    )

    def din(name, shape, dtype=F32):
        return nc.dram_tensor(name, list(shape), dtype, kind="ExternalInput").ap()

    mem_plus = din("mem_plus", (MEMP, V))
    timings = din("timings", (B, M))
    msur = din("msur", (B, M))
    w0s = din("W0s", (W0S_ROWS, H), BF16)
    w1 = din("W1", (H, H), BF16)
    w2 = din("W2", (H, H), BF16)
    wout = din("Wout", (H, V), BF16)
    b0r = din("b0r", (B, H))
    b1r = din("b1r", (B, H))
    b2r = din("b2r", (B, H))
    boutr = din("boutr", (B, V))
    c_eye = din("c_eye", (128, 128))
    c_esel = din("c_esel", (B, B * 128))
    c_iota = din("c_iota512", (B, M))
    c_iotam = din("c_iotam", (128, 4))
    c_rrow = din("c_rrow", (128, RPC))
    c_sel16 = din("c_sel16", (1, 16 * 128))
    c_amask = din("c_amask", (128, 128))
    c_coff = din("c_coff", (128, 128))
    c_rtd = din("c_rtd", (RPC, NBT * TD * 128))

    out = nc.dram_tensor("out", [B, V], F32, kind="ExternalOutput").ap()
    dbg = (nc.dram_tensor("dbg", [128, 256], F32, kind="ExternalOutput").ap()
           if stage != "full" else None)

    with tile.TileContext(nc) as tc:
        with (
            tc.tile_pool(name="const", bufs=1) as constp,
            tc.tile_pool(name="state", bufs=1) as state,
            tc.tile_pool(name="wres", bufs=1) as wres,
            tc.tile_pool(name="krep", bufs=2) as krepp,
            tc.tile_pool(name="pt", bufs=8) as ptp,
            tc.tile_pool(name="w0t", bufs=12) as w0p,
            tc.tile_pool(name="pk", bufs=1, space="PSUM") as pkp,
            tc.tile_pool(name="pflat", bufs=1, space="PSUM") as pflatp,
            tc.tile_pool(name="psort", bufs=1, space="PSUM") as psortp,
            tc.tile_pool(name="ptr", bufs=2, space="PSUM") as ptrp,
            tc.tile_pool(name="ph", bufs=1, space="PSUM") as php,
            tc.tile_pool(name="dram", bufs=1, space="DRAM") as dramp,
        ):
            # ---- constants / small state into SBUF
            def load(pool, ap):
                t = pool.tile(list(ap.shape), ap.dtype, tag=f"ld_{ap.tensor.name}")
                nc.sync.dma_start(t[:], ap)
                return t

            eye = load(constp, c_eye)
            esel = load(constp, c_esel)
            iota = load(constp, c_iota)
            iotam = load(constp, c_iotam)
            rrow = load(constp, c_rrow)
            sel16 = load(constp, c_sel16)
            amask = load(constp, c_amask)
            coff = load(constp, c_coff)
            rtd = load(constp, c_rtd)
            b0s = load(constp, b0r)
            b1s = load(constp, b1r)
            b2s = load(constp, b2r)
            bouts = load(constp, boutr)
            t_sb = load(state, timings)
            ms_sb = load(state, msur)

            # resident tail-layer weights (bf16, loaded once up front)
            wos = wres.tile([128, 8 * V], BF16, tag="wos")
            w1s_sb = wres.tile([128, 8 * H], BF16, tag="w1s")
            w2s_sb = wres.tile([128, 8 * H], BF16, tag="w2s")
            for kt in range(8):
                nc.sync.dma_start(wos[:, kt * V:(kt + 1) * V], wout[kt * 128:(kt + 1) * 128, :])
                nc.sync.dma_start(w1s_sb[:, kt * H:(kt + 1) * H], w1[kt * 128:(kt + 1) * 128, :])
                nc.sync.dma_start(w2s_sb[:, kt * H:(kt + 1) * H], w2[kt * 128:(kt + 1) * 128, :])

            # ---- stage A: keys -------------------------------------------
            msur2 = state.tile([B, M], F32, tag="msur2")
            nc.vector.tensor_scalar(msur2[:], ms_sb[:], 0.9, None, ALU.mult)
            minv = state.tile([B, 1], F32, tag="minv")
            nc.vector.tensor_reduce(minv[:], msur2[:], axis=mybir.AxisListType.X, op=ALU.min)
            mask = state.tile([B, M], mybir.dt.uint8, tag="mask")
            nc.vector.tensor_scalar(mask[:], msur2[:], minv[:], None, ALU.is_equal)
            cand = state.tile([B, M], F32, tag="cand")
            nc.vector.memset(cand[:], 1.0e9)
            nc.vector.copy_predicated(cand[:], mask[:], iota[:])
            idx0 = state.tile([B, 1], F32, tag="idx0")
            nc.vector.tensor_reduce(idx0[:], cand[:], axis=mybir.AxisListType.X, op=ALU.min)

            keys = state.tile([B, M], F32, tag="keys")
            # (t+1)*512 + m  =  t*512 + 512 + m
            nc.vector.tensor_scalar(keys[:], t_sb[:], 512.0, 512.0, ALU.mult, ALU.add)
            nc.vector.tensor_tensor(keys[:], keys[:], iota[:], ALU.add)
            mask2 = state.tile([B, M], mybir.dt.uint8, tag="mask2")
            nc.vector.tensor_scalar(mask2[:], iota[:], idx0[:], None, ALU.is_equal)
            nc.vector.copy_predicated(keys[:], mask2[:], iota[:])

            # ---- keysT via PE transpose ----------------------------------
            keysT = state.tile([128, 4 * B], F32, tag="keysT")
            for mt in range(4):
                ptt = ptrp.tile([128, 128], F32, tag="pm")
                nc.tensor.transpose(ptt[:, 0:B], keys[:, mt * 128:(mt + 1) * 128], eye[0:B, 0:B])
                nc.scalar.activation(keysT[:, mt * B:(mt + 1) * B], ptt[:, 0:B], ACTF.Copy)

            # ---- ranks, P^T, order/sorted extraction ---------------------
            rank_sb = state.tile([128, 4 * B], F32, tag="rank")
            scratch = state.tile([128, M], F32, tag="scratch")
            flat = state.tile([1, NIDX], F32, tag="flat")
            psort_t = psortp.tile([RPC, B], F32, tag="psort")
            for g in range(4):
                pflat_t = pflatp.tile([1, 8 * RPC], F32, tag="pflat")
                for b8 in range(8):
                    b = g * 8 + b8
                    pk_t = pkp.tile([128, M], F32, tag="pkrep")
                    nc.tensor.matmul(pk_t[:], esel[:, b * 128:(b + 1) * 128], keys[:],
                                     start=True, stop=True)
                    krep = krepp.tile([128, M], F32, tag="krep")
                    nc.scalar.activation(krep[:], pk_t[:], ACTF.Copy)
                    for mt in range(4):
                        nc.vector.tensor_scalar(
                            scratch[:], krep[:], keysT[:, mt * B + b:mt * B + b + 1], None,
                            ALU.is_lt, ALU.add,
                            accum_out=rank_sb[:, b * 4 + mt:b * 4 + mt + 1])
                    pts = []
                    for mt in range(4):
                        pt_t = ptp.tile([128, RPC], F32, tag="pt")
                        nc.vector.tensor_scalar(
                            pt_t[:], rrow[:], rank_sb[:, b * 4 + mt:b * 4 + mt + 1], None,
                            ALU.is_equal)
                        pts.append(pt_t)
                    for mt in range(4):
                        nc.tensor.matmul(
                            pflat_t[0:1, b8 * RPC:(b8 + 1) * RPC],
                            iotam[:, mt:mt + 1], pts[mt][:],
                            start=(mt == 0), stop=(mt == 3))
                        nc.tensor.matmul(
                            psort_t[0:RPC, b:b + 1],
                            pts[mt][:], keysT[:, mt * B + b:mt * B + b + 1],
                            start=(mt == 0), stop=(mt == 3))
                nc.scalar.activation(flat[0:1, g * 512:(g + 1) * 512], pflat_t[:], ACTF.Copy)

            # ---- bits from sorted keys -----------------------------------
            # binary decomposition of sorted key (< 2^19); timing bit d of t
            # is key bit d+9.  u_all[:, d*B:(d+1)*B] = bit (d+9) of key.
            skT = state.tile([RPC, B], F32, tag="skT")
            nc.scalar.activation(skT[:], psort_t[:], ACTF.Copy)
            rem = state.tile([RPC, B], F32, tag="rem")
            nc.vector.tensor_copy(rem[:], skT[:])
            u_all = state.tile([RPC, TD * B], F32, tag="u_all")
            tmpu = state.tile([RPC, B], F32, tag="tmpu")
            for j in range(18, 8, -1):
                d = j - 9
                ud = u_all[:, d * B:(d + 1) * B]
                nc.vector.tensor_scalar(ud, rem[:], float(2 ** j), None, ALU.is_ge)
                nc.vector.tensor_scalar(tmpu[:], ud, float(2 ** j), None, ALU.mult)
                nc.vector.tensor_tensor(rem[:], rem[:], tmpu[:], ALU.subtract)
            # bits_sb[t][p, b] = u_{d(p)}[r(p), b] via selection matmuls
            bits_sb = state.tile([128, NBT * B], BF16, tag="bits")
            for t in range(NBT):
                pb = ptrp.tile([128, 128], F32, tag="pm")
                for d in range(TD):
                    nc.tensor.matmul(pb[:, 0:B],
                                     rtd[:, (t * TD + d) * 128:(t * TD + d + 1) * 128],
                                     u_all[:, d * B:(d + 1) * B],
                                     start=(d == 0), stop=(d == TD - 1))
                nc.scalar.activation(bits_sb[:, t * B:(t + 1) * B], pb[:, 0:B], ACTF.Copy)

            # ---- gather indices ------------------------------------------
            pidx_t = ptrp.tile([128, 128], F32, tag="pm")
            flat_v = flat.rearrange("p (n s) -> p n s", s=16)
            for k in range(16):
                nc.tensor.matmul(pidx_t[:], sel16[0:1, k * 128:(k + 1) * 128],
                                 flat_v[:, :, k], start=(k == 0), stop=(k == 15))
            tmpidx = state.tile([128, 128], F32, tag="tmpidx")
            nc.vector.tensor_tensor(tmpidx[:], pidx_t[:], amask[:], ALU.mult)
            idx_sb = state.tile([128, 128], I16, tag="idx")
            nc.vector.tensor_tensor(idx_sb[:], tmpidx[:], coff[:], ALU.add)

            if stage == "idx":
                nc.vector.tensor_copy(tmpidx[:], idx_sb[:])
                nc.sync.dma_start(dbg[:, 0:128], tmpidx[:])
                nc.sync.dma_start(dbg[:, 128:256], bits_sb[:, 0:128])
            do_rest = lvl >= 1
            try:
              if not do_rest:
                  raise _SkipRest
              # ---- gather + transpose to pred_in^T tiles -------------------
              G = state.tile([128, 16 * V], F32, tag="G")
              nc.gpsimd.dma_gather(
                  out_ap=G.rearrange("p (c e) -> p c e", e=V),
                  in_ap=mem_plus,
                  idxs_ap=idx_sb[:],
                  num_idxs=NIDX,
                  num_idxs_reg=NIDX,
                  elem_size=V,
                  single_packet=False,
              )
              T_all = state.tile([128, 16 * V], BF16, tag="T_all")
              for c in range(16):
                  for hh in range(2):
                      off = c * V + hh * 128
                      pt2 = ptrp.tile([128, 128], F32, tag="pm")
                      nc.tensor.transpose(pt2[:], G[:, off:off + 128], eye[:])
                      nc.scalar.activation(T_all[:, off:off + 128], pt2[:], ACTF.Copy)

              # ---- repack transposed tiles to k-tile-major contiguous ------
              # T_all col = 256*cb + 128*h + 64*b2 + r  ->  TK col = 64*r + 32*h + 2*cb + b2
              TK = state.tile([128, 16 * V], BF16, tag="TK")
              t_in = T_all.rearrange("p (cb h b2 r) -> p r h cb b2", cb=16, h=2, b2=2, r=RPC)
              tk_out = TK.rearrange("p (r h cb b2) -> p r h cb b2", r=RPC, h=2, cb=16, b2=2)
              nc.vector.tensor_copy(tk_out[:], t_in[:])

              if stage == "tk":
                  nc.sync.dma_start(dbg[:, 0:256], TK[:, 0:256])
              if lvl < 2:
                  raise _SkipRest
              # ---- main matmul: partial h = pred_in_shard @ W0_shard -------
              ph_t = php.tile([B, H], F32, tag="ph")
              for kt in range(nkt_lim):
                  w0t = w0p.tile([128, H], BF16, tag="w0t")
                  nc.sync.dma_start(w0t[:], w0s[kt * 128:(kt + 1) * 128, :])
                  if kt < NKT:
                      lhsT = TK[:, kt * B:(kt + 1) * B]
                  else:
                      tb = kt - NKT
                      lhsT = bits_sb[:, tb * B:(tb + 1) * B]
                  last = kt == nkt_lim - 1
                  nc.tensor.matmul(ph_t[:, 0:512], lhsT, w0t[:, 0:512],
                                   start=(kt == 0), stop=last)
                  nc.tensor.matmul(ph_t[:, 512:1024], lhsT, w0t[:, 512:1024],
                                   start=(kt == 0), stop=last)

              # ---- AllReduce partial h over the 8 cores --------------------
              part_h = state.tile([B, H], F32, tag="part_h")
              nc.vector.tensor_copy(part_h[:], ph_t[:])
              if stage == "parth":
                  nc.sync.dma_start(dbg[0:B, 0:256], part_h[:, 0:256])
              if lvl < 3:
                  raise _SkipRest
              cc_in = dramp.tile([B, H], F32, tag="cc_in")
              cc_out = dramp.tile([B, H], F32, tag="cc_out")
              nc.sync.dma_start(cc_in[:], part_h[:])
              nc.gpsimd.collective_compute(
                  "AllReduce", ALU.add,
                  replica_groups=[list(range(NC))],
                  ins=[cc_in.opt()],
                  outs=[cc_out.opt()],
              )
              h_sb = state.tile([B, H], F32, tag="h_sb")
              nc.sync.dma_start(h_sb[:], cc_out[:])

              # ---- dense layers (replicated on every core) -----------------
              nc.vector.tensor_tensor(h_sb[:], h_sb[:], b0s[:], ALU.add)
              nc.vector.tensor_scalar(h_sb[:], h_sb[:], 0.0, None, ALU.max)

              def dense(h_in, w_sb, bias_sb, n_out, relu, tag):
                  hT = state.tile([128, 8 * B], BF16, tag=f"hT_{tag}")
                  for kt in range(8):
                      ptt = ptrp.tile([128, 128], F32, tag="pm")
                      nc.tensor.transpose(ptt[:, 0:B], h_in[:, kt * 128:(kt + 1) * 128], eye[0:B, 0:B])
                      nc.scalar.activation(hT[:, kt * B:(kt + 1) * B], ptt[:, 0:B], ACTF.Copy)
                  pho = php.tile([B, n_out], F32, tag="ph")
                  for kt in range(8):
                      for j0 in range(0, n_out, 512):
                          jn = min(512, n_out - j0)
                          rhs = w_sb[:, kt * n_out + j0:kt * n_out + j0 + jn]
                          nc.tensor.matmul(
                              pho[:, j0:j0 + jn], hT[:, kt * B:(kt + 1) * B], rhs,
                              start=(kt == 0), stop=(kt == 7))
                  h_next = state.tile([B, n_out], F32, tag=f"h_{tag}")
                  nc.vector.tensor_tensor(h_next[:], pho[:], bias_sb[:], ALU.add)
                  if relu:
                      nc.vector.tensor_scalar(h_next[:], h_next[:], 0.0, None, ALU.max)
                  return h_next

              h1 = dense(h_sb, w1s_sb, b1s, H, True, "l1")
              h2 = dense(h1, w2s_sb, b2s, H, True, "l2")
              logits = dense(h2, wos, bouts, V, False, "lo")
              nc.sync.dma_start(out, logits[:])
            except _SkipRest:
                pass

    nc.compile()
    return nc


def make_in_maps(inputs):
    x = np.asarray(inputs["x"], np.float32)
    memory = np.asarray(inputs["memory"], np.float32)
    timings = np.asarray(inputs["memory_timings"], np.float32)
    msur = np.asarray(inputs["memory_surprise"], np.float32)
    W0 = np.asarray(inputs["W0"], np.float32)
    W1 = np.asarray(inputs["W1"], np.float32)
    W2 = np.asarray(inputs["W2"], np.float32)
    Wout = np.asarray(inputs["Wout"], np.float32)
    b0 = np.asarray(inputs["b0"], np.float32)
    b1 = np.asarray(inputs["b1"], np.float32)
    b2 = np.asarray(inputs["b2"], np.float32)
    bout = np.asarray(inputs["bout"], np.float32)

    mem_plus = np.concatenate([memory.reshape(MEMROWS, V), x], 0)

    # shared constants
    eye = np.eye(128, dtype=np.float32)
    esel = np.zeros((B, B * 128), np.float32)
    for b in range(B):
        esel[b, b * 128:(b + 1) * 128] = 1.0
    iota512 = np.broadcast_to(np.arange(M, dtype=np.float32), (B, M)).copy()
    iotam = np.empty((128, 4), np.float32)
    for mt in range(4):
        iotam[:, mt] = np.arange(128) + mt * 128
    sel16 = np.zeros((1, 16 * 128), np.float32)
    for k in range(16):
        p = np.arange(128)
        sel16[0, k * 128:(k + 1) * 128] = (p % 16 == k)
    rtd = np.zeros((RPC, NBT * TD * 128), np.float32)
    for t in range(NBT):
        for p in range(128):
            l = t * 128 + p
            rp, dp = l // TD, l % TD
            rtd[rp, (t * TD + dp) * 128 + p] = 1.0

    shared = {
        "mem_plus": mem_plus,
        "timings": timings,
        "msur": msur,
        "W1": W1.astype(BF), "W2": W2.astype(BF), "Wout": Wout.astype(BF),
        "b0r": np.broadcast_to(b0, (B, H)).copy(),
        "b1r": np.broadcast_to(b1, (B, H)).copy(),
        "b2r": np.broadcast_to(b2, (B, H)).copy(),
        "boutr": np.broadcast_to(bout, (B, V)).copy(),
        "c_eye": eye, "c_esel": esel, "c_iota512": iota512,
        "c_iotam": iotam, "c_sel16": sel16, "c_rtd": rtd,
    }

    in_maps = []
    p = np.arange(128)
    f = np.arange(128)
    ii = 16 * f[None, :] + (p % 16)[:, None]   # [128,128] linear gather positions
    bb = ii // RPC
    rr = ii % RPC
    for core in range(NC):
        w0shard = np.concatenate(
            [W0[core * RPC * V:(core + 1) * RPC * V],
             W0[M * V + core * RPC * TD: M * V + (core + 1) * RPC * TD]], 0)
        rrowc = np.broadcast_to(
            np.arange(core * RPC, (core + 1) * RPC, dtype=np.float32), (128, RPC)).copy()
        am = np.ones((128, 128), np.float32)
        co = (512.0 * bb).astype(np.float32)
        if core == 0:
            r0 = rr == 0
            am[r0] = 0.0
            co[r0] = (MEMROWS + bb)[r0]
        m = dict(shared)
        m["W0s"] = np.ascontiguousarray(w0shard).astype(BF)
        m["c_rrow"] = rrowc
        m["c_amask"] = am
        m["c_coff"] = co
        in_maps.append(m)
    return in_maps


_NC_CACHE = None


def kernel(**inputs) -> np.ndarray:
    global _NC_CACHE
    if _NC_CACHE is None:
        _NC_CACHE = build_program()
    nc = _NC_CACHE
    in_maps = make_in_maps(inputs)
    res = bass_utils.run_bass_kernel_spmd(nc, in_maps, core_ids=list(range(NC)))
    return np.asarray(res.results[0]["out"], np.float32)


if __name__ == "__main__":
    np.random.seed(0)
    build_program()
    print("build OK")



# revision 17
# speedup vs baseline: 2.0378x; 1.5770x over previous
"""Trainium2 Bass kernel for nn_Net_71270687310327 (scatter_memory).

Computation (see reference):
  - t_eff = timings+1, with argmin(surprise*0.9) slot's t_eff overridden to 0.
  - key[b,m] = 512*t_eff[b,m] + m  (distinct; stable-sort order of reference)
  - rank[b,m] = #{m' : key[b,m'] < key[b,m]}
  - pred_in = [rank-sorted memory rows | timing bits of sorted t], 4-layer MLP.

Device algorithm (per core, SPMD over 8 cores):
  - D[b][m,m'] = key[b,m]-key[b,m'] via K=4 fp16 matmuls (exact in fp32 PSUM).
  - rank = count(D>0): DVE is_gt+accum on half the tiles, ACT Sign+accum on the
    other half ((acc+511)/2 fixup later).
  - local_scatter with idx = rank - 64*core inverts the permutation for this
    core's 64 ranks (negative idxs ignored); gives sorted_m / sorted_t.
  - 4 gather waves (16 ranks x 32 batch rows each) -> PE transpose -> repack to
    pred_in^T k-tiles (bf16); timing-bit k-tiles from sorted_t.
  - h_partial = pred_in_shard @ W0_shard (bf16, 133 k-tiles), AllReduce,
    replicated W1/W2/Wout tail (bf16, SBUF-resident).
W0 row-shard per core: 64 ranks * (256 mem rows + 10 bit rows), host-permuted
to match on-device tile layouts.
"""

import sys, os

sys.path.insert(0, "/opt/trn_rl_repo")

import numpy as np
import ml_dtypes
BF = ml_dtypes.bfloat16

import concourse.bass as bass
import concourse.bacc as bacc
import concourse.mybir as mybir
from concourse import tile
from concourse import bass_utils

F32 = mybir.dt.float32
BF16 = mybir.dt.bfloat16
FP16 = mybir.dt.float16
I16 = mybir.dt.int16
U8 = mybir.dt.uint8
ALU = mybir.AluOpType
ACTF = mybir.ActivationFunctionType

B, M, V, H, TD = 32, 512, 256, 1024, 10
NC = 8
RPC = M // NC            # 64 ranks per core
MEMROWS = B * M          # 16384
MEMP = MEMROWS + B       # 16416 (gather source rows: memory rows + x rows)
NKT = RPC * V // 128     # 128 mem k-tiles per core
NBT = RPC * TD // 128    # 5 bits k-tiles per core
W0S_ROWS = RPC * V + RPC * TD  # 17024
NW = 4                   # gather waves
WR = RPC // NW           # 16 ranks per wave
W0BUFS = 38              # W0 prefetch ring tiles (256KB bf16 each)


def build_program():
    nc = bacc.Bacc(
        "TRN2",
        target_bir_lowering=False,
        debug=False,
        enable_asserts=False,
        num_devices=NC,
    )

    def din(name, shape, dtype=F32):
        return nc.dram_tensor(name, list(shape), dtype, kind="ExternalInput").ap()

    mem_plus = din("mem_plus", (MEMP, V))
    timings = din("timings", (B, M))
    msur = din("msur", (B, M))
    w0s = din("W0s", (W0S_ROWS, H), BF16)
    w1 = din("W1", (H, H), BF16)
    w2 = din("W2", (H, H), BF16)
    wout = din("Wout", (H, V), BF16)
    b0r = din("b0r", (B, H))
    b1r = din("b1r", (B, H))
    b2r = din("b2r", (B, H))
    boutr = din("boutr", (B, V))
    c_eye = din("c_eye", (128, 128))
    c_eye16 = din("c_eye16", (128, 128), FP16)
    c_esel512 = din("c_esel512", (B, B * 128), FP16)
    c_rep16 = din("c_rep16", (WR, 128), FP16)
    c_esel1 = din("c_esel1", (B, B * 128), FP16)
    c_iota = din("c_iota512", (B, M))
    c_base = din("c_base", (B, 1))      # 64*core
    c_fixa = din("c_fixa", (B, 1))      # 255.5 - 64*core
    c_ovr = din("c_ovr", (128, B), I16)   # core0: x-row gather index override
    c_ovrm = din("c_ovrm", (128, B), U8)  # core0: override mask (p%16==0)

    out = nc.dram_tensor("out", [B, V], F32, kind="ExternalOutput").ap()

    with tile.TileContext(nc) as tc:
        with (
            tc.tile_pool(name="const", bufs=1) as constp,
            tc.tile_pool(name="state", bufs=1) as state,
            tc.tile_pool(name="wres", bufs=1) as wres,
            tc.tile_pool(name="gw", bufs=2) as gwp,
            tc.tile_pool(name="tw", bufs=2) as twp,
            tc.tile_pool(name="w0t", bufs=W0BUFS) as w0p,
            tc.tile_pool(name="pk", bufs=3, space="PSUM") as pkp,
            tc.tile_pool(name="ptr", bufs=2, space="PSUM") as ptrp,
            tc.tile_pool(name="ptr16", bufs=1, space="PSUM") as ptr16p,
            tc.tile_pool(name="ph", bufs=1, space="PSUM") as php,
            tc.tile_pool(name="dram", bufs=1, space="DRAM") as dramp,
        ):
            def load(pool, ap):
                t = pool.tile(list(ap.shape), ap.dtype, tag=f"ld_{ap.tensor.name}")
                nc.sync.dma_start(t[:], ap)
                return t

            eye = load(constp, c_eye)
            eye16 = load(constp, c_eye16)
            esel512 = load(constp, c_esel512)
            rep16 = load(constp, c_rep16)
            esel1 = load(constp, c_esel1)
            iota = load(constp, c_iota)
            base32 = load(constp, c_base)
            fixa = load(constp, c_fixa)
            ovr = load(constp, c_ovr)
            ovrm = load(constp, c_ovrm)
            b0s = load(constp, b0r)
            b1s = load(constp, b1r)
            b2s = load(constp, b2r)
            bouts = load(constp, boutr)
            t_sb = load(state, timings)
            ms_sb = load(state, msur)

            # resident tail-layer weights (bf16)
            wos = wres.tile([128, 8 * V], BF16, tag="wos")
            w1s_sb = wres.tile([128, 8 * H], BF16, tag="w1s")
            w2s_sb = wres.tile([128, 8 * H], BF16, tag="w2s")
            for kt in range(8):
                nc.sync.dma_start(wos[:, kt * V:(kt + 1) * V], wout[kt * 128:(kt + 1) * 128, :])
                nc.sync.dma_start(w1s_sb[:, kt * H:(kt + 1) * H], w1[kt * 128:(kt + 1) * 128, :])
                nc.sync.dma_start(w2s_sb[:, kt * H:(kt + 1) * H], w2[kt * 128:(kt + 1) * 128, :])

            # W0 stream: issue every k-tile DMA up-front into the prefetch
            # ring, bits tiles first (their matmuls run first on the PE)
            w0tiles = {}
            for kt in list(range(NKT, NKT + NBT)) + list(range(NKT)):
                t = w0p.tile([128, H], BF16, tag="w0t")
                nc.sync.dma_start(t[:], w0s[kt * 128:(kt + 1) * 128, :])
                w0tiles[kt] = t

            # ---- A: t_eff & argmin override ------------------------------
            msur2 = state.tile([B, M], F32, tag="msur2")
            nc.vector.tensor_scalar(msur2[:], ms_sb[:], 0.9, None, ALU.mult)
            minv = state.tile([B, 1], F32, tag="minv")
            nc.vector.tensor_reduce(minv[:], msur2[:], axis=mybir.AxisListType.X, op=ALU.min)
            mask = state.tile([B, M], U8, tag="mask")
            nc.vector.tensor_scalar(mask[:], msur2[:], minv[:], None, ALU.is_equal)
            cand = state.tile([B, M], F32, tag="cand")
            nc.vector.memset(cand[:], 1.0e9)
            nc.vector.copy_predicated(cand[:], mask[:], iota[:])
            idx0 = state.tile([B, 1], F32, tag="idx0")
            nc.vector.tensor_reduce(idx0[:], cand[:], axis=mybir.AxisListType.X, op=ALU.min)

            t_eff = state.tile([B, M], F32, tag="t_eff")
            nc.vector.tensor_scalar(t_eff[:], t_sb[:], 1.0, None, ALU.add)
            mask2 = state.tile([B, M], U8, tag="mask2")
            nc.vector.tensor_scalar(mask2[:], iota[:], idx0[:], None, ALU.is_equal)
            zeros = state.tile([B, M], F32, tag="zeros")
            nc.vector.memset(zeros[:], 0.0)
            nc.vector.copy_predicated(t_eff[:], mask2[:], zeros[:])
            t16 = state.tile([B, M], FP16, tag="t16")
            nc.vector.tensor_copy(t16[:], t_eff[:])

            # ---- B: keys (f32) and keysT ---------------------------------
            # key[b,m] = 512*t_eff[b,m] + m  (exact f32, < 2^19)
            keys = state.tile([B, M], F32, tag="keys")
            nc.vector.tensor_scalar(keys[:], t_eff[:], 512.0, None, ALU.mult)
            nc.vector.tensor_tensor(keys[:], keys[:], iota[:], ALU.add)
            keysT = state.tile([128, 4 * B], F32, tag="keysT")
            ptt = ptrp.tile([128, 512], F32, tag="pt")
            for mt in range(4):
                nc.tensor.transpose(ptt[:, mt * B:(mt + 1) * B],
                                    keys[:, mt * 128:(mt + 1) * 128], eye[0:B, 0:B])
            nc.scalar.activation(keysT[:], ptt[:, 0:4 * B], ACTF.Copy)

            # iota_m16 used as the m-part rhs of the krep matmul + scatter data
            iota_m16 = state.tile([B, M], FP16, tag="iota_m16")
            nc.gpsimd.iota(iota_m16[:], [[1, M]], base=0, channel_multiplier=0,
                           allow_small_or_imprecise_dtypes=True)

            # ---- E: krep matmuls + rank counts ---------------------------
            # krep[p, m'] = key[b, m'] (all 128 partitions) via two exact fp16
            # selector matmuls: 512*t (esel512) + m (esel1).
            rank_dve = state.tile([128, 2 * B], F32, tag="rank_dve")
            rank_act = state.tile([128, 2 * B], F32, tag="rank_act")
            scr_d = state.tile([128, M], BF16, tag="scr_d")
            scr_a = state.tile([128, M], BF16, tag="scr_a")
            for b in range(B):
                pk = pkp.tile([128, M], F32, tag="D")
                nc.tensor.matmul(pk[:], esel512[:, b * 128:(b + 1) * 128], t16[:],
                                 start=True, stop=False)
                nc.tensor.matmul(pk[:], esel1[:, b * 128:(b + 1) * 128], iota_m16[:],
                                 start=False, stop=True)
                for mt in range(4):
                    kcol = keysT[:, mt * B + b:mt * B + b + 1]
                    if mt < 2:
                        nc.vector.tensor_scalar(
                            scr_d[:], pk[:], kcol, None, ALU.is_lt, ALU.add,
                            accum_out=rank_dve[:, mt * B + b:mt * B + b + 1])
                    else:
                        nc.scalar.activation(
                            scr_a[:], pk[:], ACTF.Sign, bias=kcol, scale=-1.0,
                            accum_out=rank_act[:, (mt - 2) * B + b:(mt - 2) * B + b + 1])

            # ---- F: rank -> rank_rel (int16, natural [B, M] layout) ------
            rank_rel = state.tile([B, M], I16, tag="rank_rel")
            for mt in range(4):
                ptf = ptrp.tile([128, 512], F32, tag="pt")
                src = rank_dve if mt < 2 else rank_act
                col = (mt % 2) * B
                nc.tensor.transpose(ptf[0:B, 0:128], src[:, col:col + B], eye[:])
                if mt < 2:
                    nc.vector.tensor_scalar(
                        rank_rel[:, mt * 128:(mt + 1) * 128], ptf[0:B, 0:128],
                        base32[:], None, ALU.subtract)
                else:
                    nc.vector.tensor_scalar(
                        rank_rel[:, mt * 128:(mt + 1) * 128], ptf[0:B, 0:128],
                        0.5, fixa[:], ALU.mult, ALU.add)

            # ---- G: invert permutation via local_scatter -----------------
            sorted_m = state.tile([B, M], FP16, tag="sorted_m")
            sorted_t = state.tile([B, M], FP16, tag="sorted_t")
            nc.gpsimd.local_scatter(sorted_m[:], iota_m16[:], rank_rel[:], B, M, M)
            nc.gpsimd.local_scatter(sorted_t[:], t16[:], rank_rel[:], B, M, M)

            # ---- H: timing-bit k-tiles (bf16) ----------------------------
            # btile[64*j + r, tb*B + b] = bit(2*tb+j) of sorted_t[b, r]
            # sorted_t^T replicated into both partition halves so every DVE op
            # stays partition-aligned.
            pts = ptr16p.tile([128, 512], FP16, tag="pt16")
            nc.tensor.transpose(pts[0:RPC, 0:B], sorted_t[:, 0:RPC], eye16[0:B, 0:B])
            nc.tensor.transpose(pts[RPC:128, B:2 * B], sorted_t[:, 0:RPC], eye16[0:B, 0:B])
            rem = state.tile([128, B], F32, tag="rem")
            nc.scalar.activation(rem[0:RPC, :], pts[0:RPC, 0:B], ACTF.Copy)
            nc.scalar.activation(rem[RPC:128, :], pts[RPC:128, B:2 * B], ACTF.Copy)
            btile = state.tile([128, NBT * B], BF16, tag="bits")
            u_s = state.tile([128, B], F32, tag="u_s")
            for d in range(TD - 1, -1, -1):
                tb, j = d // 2, d % 2
                nc.vector.tensor_scalar(u_s[:], rem[:], float(2 ** d), None, ALU.is_ge)
                nc.vector.tensor_copy(btile[64 * j:64 * j + 64, tb * B:(tb + 1) * B],
                                      u_s[64 * j:64 * j + 64, :])
                nc.vector.scalar_tensor_tensor(rem[:], u_s[:], -float(2 ** d), rem[:],
                                               ALU.mult, ALU.add)

            # ---- I: gather-wave idx tiles --------------------------------
            # sorted_m^T slice -> replicate to all 8 16-partition blocks via a
            # selector matmul (REP[u,p] = [p%16 == u]), then add 512*b.
            coff512 = state.tile([128, B], F32, tag="coff512")
            nc.gpsimd.iota(coff512[:], [[512, B]], base=0, channel_multiplier=0,
                           allow_small_or_imprecise_dtypes=True)
            idxws = []
            for w in range(NW):
                pti = ptr16p.tile([128, 512], FP16, tag="pt16")
                nc.tensor.transpose(pti[0:WR, 0:B], sorted_m[:, WR * w:WR * (w + 1)],
                                    eye16[0:B, 0:B])
                smT = state.tile([WR, B], FP16, tag="smT")
                nc.scalar.activation(smT[:], pti[0:WR, 0:B], ACTF.Copy)
                prep = ptrp.tile([128, 512], F32, tag="pt")
                nc.tensor.matmul(prep[:, 0:B], rep16[:], smT[:], start=True, stop=True)
                idxw = state.tile([128, B], I16, tag=f"idx{w}")
                nc.vector.tensor_tensor(idxw[:], prep[:, 0:B], coff512[:], ALU.add)
                if w == 0:
                    nc.vector.copy_predicated(idxw[:], ovrm[:], ovr[:])
                idxws.append(idxw)

            # ---- J: waves (gather -> transpose -> repack -> matmul) ------
            ph = php.tile([B, H], F32, tag="ph")
            # bits k-tiles first (ready earliest): kt NKT..NKT+NBT-1
            for i in range(NBT):
                lhsT = btile[:, i * B:(i + 1) * B]
                w0t = w0tiles[NKT + i]
                nc.tensor.matmul(ph[:, 0:512], lhsT, w0t[:, 0:512],
                                 start=(i == 0), stop=False)
                nc.tensor.matmul(ph[:, 512:1024], lhsT, w0t[:, 512:1024],
                                 start=(i == 0), stop=False)
            for w in range(NW):
                Gw = gwp.tile([128, 4 * V], F32, tag="G")
                nc.gpsimd.dma_gather(
                    out_ap=Gw.rearrange("p (c e) -> p c e", e=V),
                    in_ap=mem_plus,
                    idxs_ap=idxws[w][:],
                    num_idxs=M,
                    num_idxs_reg=M,
                    elem_size=V,
                    single_packet=False,
                )
                Tw = twp.tile([128, 1024], BF16, tag="T")
                for half in range(2):
                    ptw = ptrp.tile([128, 512], F32, tag="pt")
                    for q in range(4):
                        ch = half * 4 + q
                        nc.tensor.transpose(ptw[:, q * 128:(q + 1) * 128],
                                            Gw[:, ch * 128:(ch + 1) * 128], eye[:])
                    nc.scalar.activation(Tw[:, half * 512:(half + 1) * 512], ptw[:], ACTF.Copy)
                TKw = state.tile([128, 1024], BF16, tag=f"TK{w}")
                t_in = Tw.rearrange("p (c h phi plo) -> p c h phi plo",
                                    c=4, h=2, phi=8, plo=16)
                t_out = TKw.rearrange("p (plo h c phi) -> p c h phi plo",
                                      plo=16, h=2, c=4, phi=8)
                nc.vector.tensor_copy(t_out, t_in)
                for i in range(32):
                    kt = 32 * w + i
                    lhsT = TKw[:, i * B:(i + 1) * B]
                    w0t = w0tiles[kt]
                    last = kt == NKT - 1
                    nc.tensor.matmul(ph[:, 0:512], lhsT, w0t[:, 0:512],
                                     start=False, stop=last)
                    nc.tensor.matmul(ph[:, 512:1024], lhsT, w0t[:, 512:1024],
                                     start=False, stop=last)

            # ---- K: AllReduce partial h (bf16) ---------------------------
            part_h = state.tile([B, H], BF16, tag="part_h")
            nc.vector.tensor_copy(part_h[:], ph[:])
            cc_in = dramp.tile([B, H], BF16, tag="cc_in")
            cc_out = dramp.tile([B, H], BF16, tag="cc_out")
            nc.sync.dma_start(cc_in[:], part_h[:])
            nc.gpsimd.collective_compute(
                "AllReduce", ALU.add,
                replica_groups=[list(range(NC))],
                ins=[cc_in.opt()],
                outs=[cc_out.opt()],
            )
            h_bf = state.tile([B, H], BF16, tag="h_bf")
            nc.sync.dma_start(h_bf[:], cc_out[:])
            h_sb = state.tile([B, H], F32, tag="h_sb")
            nc.vector.tensor_tensor(h_sb[:], h_bf[:], b0s[:], ALU.add)
            nc.vector.tensor_scalar(h_sb[:], h_sb[:], 0.0, None, ALU.max)

            # ---- L: dense tail (replicated, bf16 resident weights) -------
            def dense(h_in, w_sb, bias_sb, n_out, relu, tag):
                hT = state.tile([128, 8 * B], BF16, tag=f"hT_{tag}")
                for g in range(2):
                    ptd = ptrp.tile([128, 512], F32, tag="pt")
                    for kt4 in range(4):
                        kt = g * 4 + kt4
                        nc.tensor.transpose(ptd[:, kt4 * B:(kt4 + 1) * B],
                                            h_in[:, kt * 128:(kt + 1) * 128], eye[0:B, 0:B])
                    nc.scalar.activation(hT[:, g * 4 * B:(g + 1) * 4 * B],
                                         ptd[:, 0:4 * B], ACTF.Copy)
                pho_full = php.tile([B, H], F32, tag="ph")
                pho = pho_full[:, 0:n_out]
                for kt in range(8):
                    for j0 in range(0, n_out, 512):
                        jn = min(512, n_out - j0)
                        rhs = w_sb[:, kt * n_out + j0:kt * n_out + j0 + jn]
                        nc.tensor.matmul(
                            pho[:, j0:j0 + jn], hT[:, kt * B:(kt + 1) * B], rhs,
                            start=(kt == 0), stop=(kt == 7))
                h_next = state.tile([B, n_out], F32, tag=f"h_{tag}")
                nc.vector.tensor_tensor(h_next[:], pho[:], bias_sb[:], ALU.add)
                if relu:
                    nc.vector.tensor_scalar(h_next[:], h_next[:], 0.0, None, ALU.max)
                return h_next

            h1 = dense(h_sb, w1s_sb, b1s, H, True, "l1")
            h2 = dense(h1, w2s_sb, b2s, H, True, "l2")
            logits = dense(h2, wos, bouts, V, False, "lo")
            nc.sync.dma_start(out, logits[:])

    nc.compile()
    return nc


def make_in_maps(inputs):
    x = np.asarray(inputs["x"], np.float32)
    memory = np.asarray(inputs["memory"], np.float32)
    timings = np.asarray(inputs["memory_timings"], np.float32)
    msur = np.asarray(inputs["memory_surprise"], np.float32)
    W0 = np.asarray(inputs["W0"], np.float32)
    W1 = np.asarray(inputs["W1"], np.float32)
    W2 = np.asarray(inputs["W2"], np.float32)
    Wout = np.asarray(inputs["Wout"], np.float32)
    b0 = np.asarray(inputs["b0"], np.float32)
    b1 = np.asarray(inputs["b1"], np.float32)
    b2 = np.asarray(inputs["b2"], np.float32)
    bout = np.asarray(inputs["bout"], np.float32)

    mem_plus = np.concatenate([memory.reshape(MEMROWS, V), x], 0)
    eye = np.eye(128, dtype=np.float32)
    esel1 = np.zeros((B, B * 128), np.float16)
    for b in range(B):
        esel1[b, b * 128:(b + 1) * 128] = 1.0
    esel512 = esel1 * 512.0

    shared = {
        "mem_plus": mem_plus,
        "timings": timings,
        "msur": msur,
        "W1": W1.astype(BF), "W2": W2.astype(BF), "Wout": Wout.astype(BF),
        "b0r": np.broadcast_to(b0, (B, H)).copy(),
        "b1r": np.broadcast_to(b1, (B, H)).copy(),
        "b2r": np.broadcast_to(b2, (B, H)).copy(),
        "boutr": np.broadcast_to(bout, (B, V)).copy(),
        "c_eye": eye,
        "c_eye16": eye.astype(np.float16),
        "c_esel512": esel512,
        "c_rep16": (np.arange(128)[None, :] % 16 == np.arange(16)[:, None]).astype(np.float16),
        "c_esel1": esel1,
        "c_iota512": np.broadcast_to(np.arange(M, dtype=np.float32), (B, M)).copy(),
    }

    W0mem = W0[:M * V]            # [M*V, H], row m*V + v
    W0bit = W0[M * V:]            # [M*TD, H], row m*TD + d
    in_maps = []
    for core in range(NC):
        # mem part: ranks 64c..64c+64, v-major (contiguous rows)
        w0m = W0mem[core * RPC * V:(core + 1) * RPC * V]
        # bits part: tile tb, partition p -> rank (64c + p%64), bit (2*tb + p//64)
        p = np.arange(128)
        rows = []
        for tb in range(NBT):
            gl = (core * RPC + (p % 64)) * TD + (2 * tb + p // 64)
            rows.append(W0bit[gl])
        w0b = np.concatenate(rows, 0)
        w0shard = np.concatenate([w0m, w0b], 0)

        ovr = np.zeros((128, B), np.int16)
        ovrm = np.zeros((128, B), np.uint8)
        if core == 0:
            ovr[0::16, :] = (MEMROWS + np.arange(B)).astype(np.int16)[None, :]
            ovrm[0::16, :] = 1
        m = dict(shared)
        m["W0s"] = np.ascontiguousarray(w0shard).astype(BF)
        m["c_base"] = np.full((B, 1), 64.0 * core, np.float32)
        m["c_fixa"] = np.full((B, 1), 255.5 - 64.0 * core, np.float32)
        m["c_ovr"] = ovr
        m["c_ovrm"] = ovrm
        in_maps.append(m)
    return in_maps


_NC_CACHE = None


def kernel(**inputs) -> np.ndarray:
    global _NC_CACHE
    if _NC_CACHE is None:
        _NC_CACHE = build_program()
    nc = _NC_CACHE
    in_maps = make_in_maps(inputs)
    res = bass_utils.run_bass_kernel_spmd(nc, in_maps, core_ids=list(range(NC)))
    return np.asarray(res.results[0]["out"], np.float32)


if __name__ == "__main__":
    np.random.seed(0)
    build_program()
    print("build OK")
